# revision 2
# baseline (speedup 1.0000x reference)
"""Trainium2 Bass kernel for nn_GetNodeK (gnn_message_passing).

out[b,i,n,m,:] = node_embedding[b, nbr_idx[b, nbr_idx[b,i,n], m], :]

Sharding: data-parallel over B (8 batches -> 8 cores, one batch per core).

Let nbr_flat = nbr_idx[b].reshape(6144) (values < 256) and define the
one-hop table G[j] = concat_m emb[nbr[j,m]] (256 rows x 12 KB = 3.1 MB).
Then out[b, k=(i*24+n)] = G[nbr_flat[k]] -- the 2-hop gather factors into
two index-driven stages that both use the raw nbr values (no chained
index arithmetic anywhere).

v2 (default): stage 1 dma_gather emb->G in SBUF (permuted so scatter-token
j sits at partition j%128, half j//128, 12 KB contiguous); stage 2 is
T = max_j count(j) rounds of indirect_dma_start scatter SBUF->DRAM where
round r writes G[j] to the r-th output row that references j (OOB-skip
via bounds_check for exhausted tokens). HBM traffic: 75.5 MB write +
3.1 MB read per core (roofline-ish).

v1 (fallback): stage 1 gather -> G -> DRAM; stage 2 dma_gather 12 KB rows
from G_dram -> SBUF tiles -> sequential DMA out. Extra 75.5 MB read.
"""
import numpy as np

from concourse import bass, bacc, mybir
import concourse.tile as tile
from concourse.bass_utils import run_bass_kernel_spmd

B, At, Nbr, F = 8, 256, 24, 128
NI = At * Nbr        # 6144 indices per batch
ROW = Nbr * F        # 3072 f32 = 12 KB per stage-2 row
CH = 512             # v1 stage-2 chunk (indices per gather call)
NCHUNK = NI // CH    # 12
OOB = 8192           # idx sentinel > NI-1 -> skipped by bounds_check

VERSION = "v3"
_CACHED = {}


# ---------------------------------------------------------------- v1 ----
def _build_nc_v1():
    nc = bacc.Bacc("TRN2", target_bir_lowering=False, debug=False)
    emb = nc.dram_tensor("emb", [At, F], mybir.dt.float32, kind="ExternalInput")
    gidx = nc.dram_tensor("gidx", [128, NI // 16], mybir.dt.int16, kind="ExternalInput")
    g_dram = nc.dram_tensor("g_scratch", [NI, F], mybir.dt.float32)
    out = nc.dram_tensor("out", [NI, ROW], mybir.dt.float32, kind="ExternalOutput")

    with tile.TileContext(nc) as tc:
        with tc.tile_pool(name="pool0", bufs=1) as pool0, \
             tc.tile_pool(name="pool2", bufs=2) as pool2:
            idx_t = pool0.tile([128, NI // 16], mybir.dt.int16)
            nc.sync.dma_start(idx_t[:], gidx[:])

            g_t = pool0.tile([128, NI // 128, F], mybir.dt.float32)
            nc.gpsimd.dma_gather(g_t[:], emb[:], idx_t[:], NI, NI, F,
                                 single_packet=False)
            nc.sync.dma_start(
                g_dram[:].rearrange("(s p) e -> p s e", p=128), g_t[:]
            )

            g_view = g_dram[:].rearrange("(j k) e -> j (k e)", k=Nbr)  # [256, 3072]
            for c in range(NCHUNK):
                t2 = pool2.tile([128, CH // 128, ROW], mybir.dt.float32, tag="t2")
                nc.gpsimd.dma_gather(
                    t2[:], g_view,
                    idx_t[:, c * (CH // 16):(c + 1) * (CH // 16)],
                    CH, CH, ROW,
                )
                nc.sync.dma_start(
                    out[c * CH:(c + 1) * CH].rearrange("(s p) e -> p s e", p=128),
                    t2[:],
                )
    nc.compile()
    return nc


def _prep_v1(nbr16_b):
    flat = nbr16_b.reshape(-1)
    return {"gidx": np.tile(flat.reshape(NI // 16, 16).T, (8, 1))}


# ---------------------------------------------------------------- v2 ----
_T_PERM = None


def _v1_perm():
    """idx1[t] = nbr[(t//128//24)*128 + t%128, (t//128)%24] as flat index."""
    global _T_PERM
    if _T_PERM is None:
        t = np.arange(NI)
        s, p = t // 128, t % 128
        j, m = (s // Nbr) * 128 + p, s % Nbr
        _T_PERM = j * Nbr + m
    return _T_PERM


def _prep_v2(nbr16_b, T):
    flat = nbr16_b.reshape(-1)
    idx1 = flat[_v1_perm()]
    gidx = np.tile(idx1.reshape(NI // 16, 16).T, (8, 1))

    counts = np.bincount(flat, minlength=At)
    order = np.argsort(flat, kind="stable")
    tbl = np.full((At, T), OOB, dtype=np.int32)
    pos = 0
    for j in range(At):
        c = counts[j]
        tbl[j, :c] = order[pos:pos + c]
        pos += c
    sidx = np.empty((128, T, 2), dtype=np.int32)
    for q in range(2):
        sidx[:, :, q] = tbl[q * 128:(q + 1) * 128, :]
    return {"gidx": gidx, "sidx": sidx}


def _build_nc_v2(T):
    nc = bacc.Bacc("TRN2", target_bir_lowering=False, debug=False)
    emb = nc.dram_tensor("emb", [At, F], mybir.dt.float32, kind="ExternalInput")
    gidx = nc.dram_tensor("gidx", [128, NI // 16], mybir.dt.int16, kind="ExternalInput")
    sidx = nc.dram_tensor("sidx", [128, T, 2], mybir.dt.int32, kind="ExternalInput")
    out = nc.dram_tensor("out", [NI, ROW], mybir.dt.float32, kind="ExternalOutput")

    with tile.TileContext(nc) as tc:
        with tc.tile_pool(name="pool0", bufs=1) as pool0:
            idx_t = pool0.tile([128, NI // 16], mybir.dt.int16)
            nc.sync.dma_start(idx_t[:], gidx[:])
            sidx_t = pool0.tile([128, T, 2], mybir.dt.int32)
            nc.sync.dma_start(sidx_t[:], sidx[:])

            g_t = pool0.tile([128, NI // 128, F], mybir.dt.float32)
            nc.gpsimd.dma_gather(g_t[:], emb[:], idx_t[:], NI, NI, F,
                                 single_packet=False)

            g_scatter = g_t[:].rearrange("p (q m) e -> p q (m e)", q=2)
            for r in range(T):
                for q in range(2):
                    nc.gpsimd.indirect_dma_start(
                        out=out[:],
                        out_offset=bass.IndirectOffsetOnAxis(
                            ap=sidx_t[:, r, q:q + 1], axis=0),
                        in_=g_scatter[:, q, :],
                        in_offset=None,
                        bounds_check=NI - 1,
                        oob_is_err=False,
                    )
    nc.compile()
    return nc


# ------------------------------------------------------------- driver ----
def _run(nc, in_maps, **kwargs):
    return run_bass_kernel_spmd(nc, in_maps, core_ids=list(range(B)), **kwargs)


def kernel(node_embedding: np.ndarray, nbr_idx: np.ndarray, _collect=None) -> np.ndarray:
    node_embedding = np.ascontiguousarray(node_embedding, dtype=np.float32)
    nbr16 = nbr_idx.astype(np.int16)  # values in [0, 256)

    if VERSION == "v1":
        if "v1" not in _CACHED:
            _CACHED["v1"] = _build_nc_v1()
        nc = _CACHED["v1"]
        in_maps = [{"emb": node_embedding[b], **_prep_v1(nbr16[b])}
                   for b in range(B)]
    else:
        T = int(max(np.bincount(nbr16[b].reshape(-1), minlength=At).max()
                    for b in range(B)))
        key = ("v2", T)
        if key not in _CACHED:
            _CACHED[key] = _build_nc_v2(T)
        nc = _CACHED[key]
        in_maps = [{"emb": node_embedding[b], **_prep_v2(nbr16[b], T)}
                   for b in range(B)]

    res = _run(nc, in_maps)
    if _collect is not None:
        _collect.append(res)
    outs = [res.results[b]["out"].reshape(At, Nbr, Nbr, F) for b in range(B)]
    return np.stack(outs, axis=0)



# revision 12
# speedup vs baseline: 1.7094x; 1.7094x over previous
"""Trainium2 Bass kernel for nn_GetNodeK (gnn_message_passing).

out[b,i,n,m,:] = node_embedding[b, nbr_idx[b, nbr_idx[b,i,n], m], :]

Sharding: data-parallel over B (8 batches -> 8 cores, one batch per core).

Let nbr_flat = nbr_idx[b].reshape(6144) (values < 256) and define the
one-hop table G[j] = concat_m emb[nbr[j,m]] (256 rows x 12 KB = 3.1 MB).
Then out[b, k=(i*24+n)] = G[nbr_flat[k]] -- the 2-hop gather factors into
two index-driven stages that both use the raw nbr values (no chained
index arithmetic anywhere).

v2 (default): stage 1 dma_gather emb->G in SBUF (permuted so scatter-token
j sits at partition j%128, half j//128, 12 KB contiguous); stage 2 is
T = max_j count(j) rounds of indirect_dma_start scatter SBUF->DRAM where
round r writes G[j] to the r-th output row that references j (OOB-skip
via bounds_check for exhausted tokens). HBM traffic: 75.5 MB write +
3.1 MB read per core (roofline-ish).

v1 (fallback): stage 1 gather -> G -> DRAM; stage 2 dma_gather 12 KB rows
from G_dram -> SBUF tiles -> sequential DMA out. Extra 75.5 MB read.
"""
import numpy as np

from concourse import bass, bacc, mybir
import concourse.tile as tile
from concourse.bass_utils import run_bass_kernel_spmd

B, At, Nbr, F = 8, 256, 24, 128
NI = At * Nbr        # 6144 indices per batch
ROW = Nbr * F        # 3072 f32 = 12 KB per stage-2 row
CH = 512             # v1 stage-2 chunk (indices per gather call)
NCHUNK = NI // CH    # 12
OOB = 8192           # idx sentinel > NI-1 -> skipped by bounds_check

VERSION = "v4"
_CACHED = {}


# ---------------------------------------------------------------- v1 ----
def _build_nc_v1():
    nc = bacc.Bacc("TRN2", target_bir_lowering=False, debug=False)
    emb = nc.dram_tensor("emb", [At, F], mybir.dt.float32, kind="ExternalInput")
    gidx = nc.dram_tensor("gidx", [128, NI // 16], mybir.dt.int16, kind="ExternalInput")
    g_dram = nc.dram_tensor("g_scratch", [NI, F], mybir.dt.float32)
    out = nc.dram_tensor("out", [NI, ROW], mybir.dt.float32, kind="ExternalOutput")

    with tile.TileContext(nc) as tc:
        with tc.tile_pool(name="pool0", bufs=1) as pool0, \
             tc.tile_pool(name="pool2", bufs=2) as pool2:
            idx_t = pool0.tile([128, NI // 16], mybir.dt.int16)
            nc.sync.dma_start(idx_t[:], gidx[:])

            g_t = pool0.tile([128, NI // 128, F], mybir.dt.float32)
            nc.gpsimd.dma_gather(g_t[:], emb[:], idx_t[:], NI, NI, F,
                                 single_packet=False)
            nc.sync.dma_start(
                g_dram[:].rearrange("(s p) e -> p s e", p=128), g_t[:]
            )

            g_view = g_dram[:].rearrange("(j k) e -> j (k e)", k=Nbr)  # [256, 3072]
            for c in range(NCHUNK):
                t2 = pool2.tile([128, CH // 128, ROW], mybir.dt.float32, tag="t2")
                nc.gpsimd.dma_gather(
                    t2[:], g_view,
                    idx_t[:, c * (CH // 16):(c + 1) * (CH // 16)],
                    CH, CH, ROW,
                )
                nc.sync.dma_start(
                    out[c * CH:(c + 1) * CH].rearrange("(s p) e -> p s e", p=128),
                    t2[:],
                )
    nc.compile()
    return nc


def _prep_v1(nbr16_b):
    flat = nbr16_b.reshape(-1)
    return {"gidx": np.tile(flat.reshape(NI // 16, 16).T, (8, 1))}


# ---------------------------------------------------------------- v2 ----
_T_PERM = None


def _v1_perm():
    """idx1[t] = nbr[(t//128//24)*128 + t%128, (t//128)%24] as flat index."""
    global _T_PERM
    if _T_PERM is None:
        t = np.arange(NI)
        s, p = t // 128, t % 128
        j, m = (s // Nbr) * 128 + p, s % Nbr
        _T_PERM = j * Nbr + m
    return _T_PERM


def _prep_v2(nbr16_b, T):
    flat = nbr16_b.reshape(-1)
    idx1 = flat[_v1_perm()]
    gidx = np.tile(idx1.reshape(NI // 16, 16).T, (8, 1))

    counts = np.bincount(flat, minlength=At)
    order = np.argsort(flat, kind="stable")
    tbl = np.full((At, T), OOB, dtype=np.int32)
    pos = 0
    for j in range(At):
        c = counts[j]
        tbl[j, :c] = order[pos:pos + c]
        pos += c
    sidx = np.empty((128, T, 2), dtype=np.int32)
    for q in range(2):
        sidx[:, :, q] = tbl[q * 128:(q + 1) * 128, :]
    return {"gidx": gidx, "sidx": sidx}


def _build_nc_v2(T):
    nc = bacc.Bacc("TRN2", target_bir_lowering=False, debug=False)
    emb = nc.dram_tensor("emb", [At, F], mybir.dt.float32, kind="ExternalInput")
    gidx = nc.dram_tensor("gidx", [128, NI // 16], mybir.dt.int16, kind="ExternalInput")
    sidx = nc.dram_tensor("sidx", [128, T, 2], mybir.dt.int32, kind="ExternalInput")
    out = nc.dram_tensor("out", [NI, ROW], mybir.dt.float32, kind="ExternalOutput")

    with tile.TileContext(nc) as tc:
        with tc.tile_pool(name="pool0", bufs=1) as pool0:
            idx_t = pool0.tile([128, NI // 16], mybir.dt.int16)
            nc.sync.dma_start(idx_t[:], gidx[:])
            sidx_t = pool0.tile([128, T, 2], mybir.dt.int32)
            nc.sync.dma_start(sidx_t[:], sidx[:])

            g_t = pool0.tile([128, NI // 128, F], mybir.dt.float32)
            nc.gpsimd.dma_gather(g_t[:], emb[:], idx_t[:], NI, NI, F,
                                 single_packet=False)

            g_scatter = g_t[:].rearrange("p (q m) e -> p q (m e)", q=2)
            for r in range(T):
                for q in range(2):
                    nc.gpsimd.indirect_dma_start(
                        out=out[:],
                        out_offset=bass.IndirectOffsetOnAxis(
                            ap=sidx_t[:, r, q:q + 1], axis=0),
                        in_=g_scatter[:, q, :],
                        in_offset=None,
                        bounds_check=NI - 1,
                        oob_is_err=False,
                    )
    nc.compile()
    return nc


# ---------------------------------------------------------------- v3 ----
def _prep_v3(nbr16_b, T):
    """Per-q-half scatter: sidx[p, q, t] = out row for t-th token of node
    j = q*128+p (OOB when t >= count[j])."""
    flat = nbr16_b.reshape(-1)
    idx1 = flat[_v1_perm()]
    gidx = np.tile(idx1.reshape(NI // 16, 16).T, (8, 1))

    counts = np.bincount(flat, minlength=At)
    order = np.argsort(flat, kind="stable")
    tbl = np.full((At, T), OOB, dtype=np.int32)
    pos = 0
    for j in range(At):
        c = counts[j]
        tbl[j, :c] = order[pos:pos + c]
        pos += c
    # tbl[j=q*128+p, t] -> sidx[p, q, t]
    sidx = np.empty((128, 2, T), dtype=np.int32)
    for q in range(2):
        sidx[:, q, :] = tbl[q * 128:(q + 1) * 128, :]
    return {"gidx": gidx, "sidx": sidx}


def _build_nc_v3(T):
    nc = bacc.Bacc("TRN2", target_bir_lowering=False, debug=False)
    emb = nc.dram_tensor("emb", [At, F], mybir.dt.float32, kind="ExternalInput")
    gidx = nc.dram_tensor("gidx", [128, NI // 16], mybir.dt.int16, kind="ExternalInput")
    sidx = nc.dram_tensor("sidx", [128, 2, T], mybir.dt.int32, kind="ExternalInput")
    out = nc.dram_tensor("out", [NI, ROW], mybir.dt.float32, kind="ExternalOutput")

    with tile.TileContext(nc) as tc:
        with tc.tile_pool(name="pool0", bufs=1) as pool0:
            idx_t = pool0.tile([128, NI // 16], mybir.dt.int16)
            nc.sync.dma_start(idx_t[:], gidx[:])
            sidx_t = pool0.tile([128, 2, T], mybir.dt.int32)
            nc.sync.dma_start(sidx_t[:], sidx[:])

            # g_t[p, s, :] = emb[nbr[j(s,p), m(s)]]; per partition the free
            # dim holds G[p] (12 KB) then G[128+p] (12 KB), contiguous.
            g_t = pool0.tile([128, NI // 128, F], mybir.dt.float32)
            nc.gpsimd.dma_gather(g_t[:], emb[:], idx_t[:], NI, NI, F,
                                 single_packet=False)

            # One scatter per q half: slot (p, t) sources partition p's
            # 12 KB row G[q*128+p] via a stride-0 middle axis (so the inner
            # AP row == one slot's payload).
            g_view = g_t[:].rearrange("p (q m) e -> p q (m e)", q=2)
            for q in range(2):
                g_bcast = g_view[:, q, :].unsqueeze(1).broadcast_to(
                    [128, T, ROW])
                nc.gpsimd.indirect_dma_start(
                    out=out[:],
                    out_offset=bass.IndirectOffsetOnAxis(
                        ap=sidx_t[:, q, :], axis=0),
                    in_=g_bcast,
                    in_offset=None,
                    bounds_check=NI - 1,
                    oob_is_err=False,
                )
    nc.compile()
    return nc


# ---------------------------------------------------------------- v4 ----
def _build_nc_v4(T):
    """Raw-bass (no TileContext): per-round indirect scatters with a single
    shared completion semaphore -> no per-call serialization chain. The
    gather is split by q half so the second half's descriptor generation
    overlaps the first half's scatter transfers."""
    nc = bacc.Bacc("TRN2", target_bir_lowering=False, debug=False,
                   detect_race_conditions=False)
    emb = nc.dram_tensor("emb", [At, F], mybir.dt.float32, kind="ExternalInput")
    gidx = nc.dram_tensor("gidx", [128, NI // 16], mybir.dt.int16, kind="ExternalInput")
    sidx = nc.dram_tensor("sidx", [128, 2, T], mybir.dt.int32, kind="ExternalInput")
    out = nc.dram_tensor("out", [NI, ROW], mybir.dt.float32, kind="ExternalOutput")

    with nc.Block() as block, \
         nc.semaphore("ld_sem") as ld_sem, \
         nc.semaphore("g_sem") as g_sem, \
         nc.semaphore("s_sem") as s_sem, \
         nc.sbuf_tensor("idx_t", [128, NI // 16], mybir.dt.int16) as idx_t, \
         nc.sbuf_tensor("sidx_t", [128, 2, T], mybir.dt.int32) as sidx_t, \
         nc.sbuf_tensor("g_t", [128, NI // 128, F], mybir.dt.float32) as g_t:

        @block.sync
        def _(sync):
            sync.dma_start(idx_t[:], gidx[:]).then_inc(ld_sem, 16)
            sync.dma_start(sidx_t[:], sidx[:]).then_inc(ld_sem, 16)

        @block.gpsimd
        def _(gpsimd):
            g_view = g_t[:].rearrange("p (q m) e -> p q (m e)", q=2)
            gpsimd.wait_ge(ld_sem, 32)
            H, HC = NI // 2, NI // 32  # idxs per half, idx-tile cols per half
            for q in range(2):
                gpsimd.dma_gather(
                    g_t[:, q * (Nbr):(q + 1) * Nbr, :], emb[:],
                    idx_t[:, q * HC:(q + 1) * HC], H, H, F,
                    single_packet=False,
                ).then_inc(g_sem, 16)
                gpsimd.wait_ge(g_sem, 16 * (q + 1))
                for r in range(T):
                    gpsimd.indirect_dma_start(
                        out=out[:],
                        out_offset=bass.IndirectOffsetOnAxis(
                            ap=sidx_t[:, q, r:r + 1], axis=0),
                        in_=g_view[:, q, :],
                        in_offset=None,
                        bounds_check=NI - 1,
                        oob_is_err=False,
                    ).then_inc(s_sem, 16)
            gpsimd.wait_ge(s_sem, 16 * 2 * T)
    nc.compile()
    return nc


# ------------------------------------------------------------- driver ----
def _run(nc, in_maps, **kwargs):
    return run_bass_kernel_spmd(nc, in_maps, core_ids=list(range(B)), **kwargs)


def kernel(node_embedding: np.ndarray, nbr_idx: np.ndarray, _collect=None) -> np.ndarray:
    node_embedding = np.ascontiguousarray(node_embedding, dtype=np.float32)
    nbr16 = nbr_idx.astype(np.int16)  # values in [0, 256)

    if VERSION == "v1":
        if "v1" not in _CACHED:
            _CACHED["v1"] = _build_nc_v1()
        nc = _CACHED["v1"]
        in_maps = [{"emb": node_embedding[b], **_prep_v1(nbr16[b])}
                   for b in range(B)]
    elif VERSION in ("v3", "v4"):
        T = int(max(np.bincount(nbr16[b].reshape(-1), minlength=At).max()
                    for b in range(B)))
        key = (VERSION, T)
        if key not in _CACHED:
            _CACHED[key] = (_build_nc_v3 if VERSION == "v3"
                            else _build_nc_v4)(T)
        nc = _CACHED[key]
        in_maps = [{"emb": node_embedding[b], **_prep_v3(nbr16[b], T)}
                   for b in range(B)]
    else:
        T = int(max(np.bincount(nbr16[b].reshape(-1), minlength=At).max()
                    for b in range(B)))
        key = ("v2", T)
        if key not in _CACHED:
            _CACHED[key] = _build_nc_v2(T)
        nc = _CACHED[key]
        in_maps = [{"emb": node_embedding[b], **_prep_v2(nbr16[b], T)}
                   for b in range(B)]

    res = _run(nc, in_maps)
    if _collect is not None:
        _collect.append(res)
    outs = [res.results[b]["out"].reshape(At, Nbr, Nbr, F) for b in range(B)]
    return np.stack(outs, axis=0)

